# revision 22
# baseline (speedup 1.0000x reference)
"""Trainium2 Bass kernel for nn_DIST_loss: mean 2D Euclidean distance loss.

reference:
    d = pred[:, :2] - target[:, :2]
    loss = sum(sqrt(d0^2 + d1^2)) / (B + 1)

Strategy (pure data parallel over 8 NeuronCores):
  - Shard pred/target along batch across 8 cores (1/8 of rows each).
  - Host negates target; per chunk the kernel DMAs pred into SBUF
    (HWDGE) then DMAs -target on top with accum_op=add (SWDGE CCE),
    so d = pred - target materializes during the load itself.
  - Per compute slice (width M, interleaved x,y pairs):
      ACT  : q = d^2                      (Square)
      DVE  : s = q_even + q_odd           (strided tensor_add)
      ACT  : sqrt(s) in place + accum_out -> per-slice partial [128,1]
  - Tree-add partials on DVE, DMA [128,1] out; host sums across
    partitions and cores, divides by (B+1).

Sync-wait discipline: every engine instruction may carry at most ONE
semaphore wait (ISA limit), so the dataflow is arranged so each op has
exactly one cross-proc dependency; tiny same-engine "carrier" copies
advance an engine's own clock where Tile would otherwise emit a second
(same-engine WAW) wait.
"""

import numpy as np

B = 8388608
N_CORES = 8
RPC = B // N_CORES            # rows per core = 1048576
P = 128
FREE_TOTAL = RPC * 2 // P     # f32 elems per partition per tensor = 16384
NCHUNK = 8                    # input chunks per tensor (DMA granularity)
CW = FREE_TOTAL // NCHUNK     # chunk width (1 MiB per DMA at NCHUNK=8)
M = 2048                      # compute slice width (interleaved elems/partition)
SLICES_PER_CHUNK = CW // M
NSLICES = NCHUNK * SLICES_PER_CHUNK
MBUFS = 8                     # buffers for the q/s compute pools (= NSLICES:
                              # no slot reuse, so no reuse waits or carriers)

_NC_CACHE = {}
LAST_RESULTS = None           # BassKernelResults of the most recent run


def _build():
    import concourse.bass as bass
    import concourse.mybir as mybir
    import concourse.tile as tile

    nc = bass.Bass(
        "TRN2",
        target_bir_lowering=False,
        debug=False,
        enable_asserts=False,
        num_devices=N_CORES,
    )
    pred = nc.dram_tensor(
        "pred", [NCHUNK, P, CW], mybir.dt.float32, kind="ExternalInput"
    )
    targ = nc.dram_tensor(
        "target", [NCHUNK, P, CW], mybir.dt.float32, kind="ExternalInput"
    )
    out = nc.dram_tensor("out", [P, 1], mybir.dt.float32, kind="ExternalOutput")

    with tile.TileContext(nc) as tc:
        with (
            tc.tile_pool(name="io", bufs=1) as io_pool,
            tc.tile_pool(name="mid", bufs=MBUFS) as mid_pool,
            tc.tile_pool(name="accp", bufs=1) as acc_pool,
        ):
            # Ring layout: preds 0-6 on HWDGE lanes 0-6, pred 7 + all accums
            # on SWDGE (accum_7's queue-head wait on DMASW0 IS its RAW wait
            # on pred_7 — same semaphore). HWDGE lane 7 stays fresh for the
            # out-DMA so it needs no queue-head wait.
            d_tiles = []
            dma_handles = []
            for c in range(NCHUNK):
                d_c = io_pool.tile([P, CW], mybir.dt.float32, tag=f"d{c}")
                d_tiles.append(d_c)
            dma_handles.append(
                nc.gpsimd.dma_start(d_tiles[NCHUNK - 1][:], pred.ap()[NCHUNK - 1])
            )
            for c in range(NCHUNK - 1):
                dma_handles.append(nc.sync.dma_start(d_tiles[c][:], pred.ap()[c]))
            for c in range(NCHUNK):
                # -target accumulates onto pred in the DMA datapath (CCE add)
                dma_handles.append(
                    nc.gpsimd.dma_start(
                        d_tiles[c][:], targ.ap()[c], accum_op=mybir.AluOpType.add
                    )
                )

            accs = []
            slice_idx = 0
            for c in range(NCHUNK):
                for j in range(SLICES_PER_CHUNK):
                    sl = slice(j * M, (j + 1) * M)
                    i = slice_idx

                    # carrier: a tiny same-engine copy advances ACT's own
                    # clock past the previous-generation sqrt, eliding the
                    # WAW waits on this slice's square and sqrt
                    if i >= MBUFS:
                        ac_t = acc_pool.tile([P, 1], mybir.dt.float32, tag=f"ac{i}")
                        hac = nc.scalar.copy(ac_t[:], accs[i - MBUFS][:])
                    else:
                        hac = None

                    # ACT: q = d^2
                    q = mid_pool.tile([P, M], mybir.dt.float32, tag="q")
                    hsqr = nc.scalar.square(q[:], d_tiles[c][:, sl])
                    if hac is not None:
                        tile.add_dep_helper(hsqr.ins, hac.ins, sync=False)

                    # DVE: s = q_even + q_odd
                    qv = q[:].rearrange("p (n two) -> p two n", two=2)
                    s = mid_pool.tile([P, M // 2], mybir.dt.float32, tag="s")
                    nc.vector.tensor_add(s[:], qv[:, 0], qv[:, 1])

                    # ACT: dist = sqrt(s) in place, accum partial
                    acc_i = acc_pool.tile([P, 1], mybir.dt.float32, tag=f"acc{i}")
                    accs.append(acc_i)
                    hsq = nc.scalar.activation(
                        s[:],
                        s[:],
                        mybir.ActivationFunctionType.Sqrt,
                        accum_out=acc_i[:],
                    )
                    slice_idx += 1

            # Tree-add the per-slice partials on DVE (unique outputs, no WAW)
            lvl = 0
            cur = accs
            final_add = None
            while len(cur) > 1:
                nxt = []
                for k in range(0, len(cur) - 1, 2):
                    r = acc_pool.tile([P, 1], mybir.dt.float32, tag=f"t{lvl}_{k}")
                    final_add = nc.vector.tensor_add(r[:], cur[k][:], cur[k + 1][:])
                    nxt.append(r)
                if len(cur) % 2:
                    nxt.append(cur[-1])
                cur = nxt
                lvl += 1

            # SP observes the final add first, so the out-DMA needs only its
            # own queue-head wait
            with nc.sync.register("tailr") as rr:
                hm = nc.sync.reg_mov(rr, 0)
                tile.add_dep_helper(
                    hm.ins, final_add.ins, sync=True, reason="SP observes final add"
                )
                hout = nc.sync.dma_start(out.ap(), cur[0][:])
                tile.add_dep_helper(
                    hout.ins, hm.ins, sync=False, reason="out-DMA after SP observer"
                )

                # SP observer chain: the kernel-tail drain inherits a wait
                # for every proc SP hasn't observed, and it too is capped at
                # one wait. One reg_mov per outstanding completion elides
                # the drain's waits.
                for h in dma_handles + [hsq, hout]:
                    hm = nc.sync.reg_mov(rr, 0)
                    tile.add_dep_helper(
                        hm.ins, h.ins, sync=True, reason="SP observes for tail drain"
                    )
    return nc


def _get_nc():
    if "nc" not in _NC_CACHE:
        _NC_CACHE["nc"] = _build()
    return _NC_CACHE["nc"]


def kernel(pred, target, **run_kwargs):
    global LAST_RESULTS
    from concourse.bass_utils import run_bass_kernel_spmd

    pred = np.ascontiguousarray(np.asarray(pred, dtype=np.float32))
    target = np.ascontiguousarray(np.asarray(target, dtype=np.float32))
    assert pred.shape == (B, 2) and target.shape == (B, 2)

    neg_target = -target
    in_maps = []
    for c in range(N_CORES):
        sl = slice(c * RPC, (c + 1) * RPC)
        in_maps.append(
            {
                "pred": pred[sl].reshape(NCHUNK, P, CW),
                "target": neg_target[sl].reshape(NCHUNK, P, CW),
            }
        )

    nc = _get_nc()
    results = run_bass_kernel_spmd(
        nc, in_maps, core_ids=list(range(N_CORES)), **run_kwargs
    )
    LAST_RESULTS = results

    total = np.float64(0.0)
    for r in results.results:
        total += r["out"].astype(np.float64).sum()
    loss = np.float32(total / np.float64(B + 1))
    return np.asarray(loss, dtype=np.float32)


# revision 27
# speedup vs baseline: 1.0712x; 1.0712x over previous
"""Trainium2 Bass kernel for nn_DIST_loss: mean 2D Euclidean distance loss.

reference:
    d = pred[:, :2] - target[:, :2]
    loss = sum(sqrt(d0^2 + d1^2)) / (B + 1)

Strategy (pure data parallel over 8 NeuronCores):
  - Shard pred/target along batch across 8 cores (1/8 of rows each).
  - Host negates target; per chunk the kernel DMAs pred into SBUF
    (HWDGE) then DMAs -target on top with accum_op=add (SWDGE CCE),
    so d = pred - target materializes during the load itself.
  - Chunks DESCEND in size: the HBM stream time is fixed by total bytes,
    so the only reducible serial tail is the last chunk's compute chain —
    a small final chunk shrinks it.
  - Per chunk (width W, interleaved x,y pairs):
      ACT  : q = d^2                      (Square)
      DVE  : s = q_even + q_odd           (strided tensor_add)
      ACT  : sqrt(s) in place + accum_out -> per-chunk partial [128,1]
  - Partials for chunks 0..6 tree-add early; one final add with the last
    chunk's partial; DMA [128,1] out; host sums across partitions and
    cores, divides by (B+1).

Sync-wait discipline: every engine instruction may carry at most ONE
semaphore wait (ISA limit). All tiles are unique (no pool-slot reuse),
so each op has exactly one cross-proc dependency. DMA ring layout:
preds 0-6 on HWDGE lanes 0-6, pred 7 first on SWDGE lane 0 followed by
the 8 accums (accum_7 wraps to lane 0, where its queue-head wait IS its
RAW wait on pred_7); the out-DMA takes fresh HWDGE lane 7.
"""

import numpy as np

B = 8388608
N_CORES = 8
RPC = B // N_CORES            # rows per core = 1048576
P = 128
FREE_TOTAL = RPC * 2 // P     # f32 elems per partition per tensor = 16384
# Descending chunk widths (elems/partition, interleaved pairs); sum = 16384.
# NOTE: accum (CCE) DMAs misbehave on HW when a per-partition contiguous run
# exceeds 2048 elements (the CCE element cap) — verified empirically: chunk
# widths > 2048 pass CoreSim but corrupt results on hardware. Keep <= 2048.
CHUNK_WIDTHS = [2048] * 8
NCHUNK = len(CHUNK_WIDTHS)
CHUNK_OFFS = [sum(CHUNK_WIDTHS[:c]) for c in range(NCHUNK)]
assert sum(CHUNK_WIDTHS) == FREE_TOTAL

_NC_CACHE = {}
LAST_RESULTS = None           # BassKernelResults of the most recent run


def _build():
    import concourse.bass as bass
    import concourse.mybir as mybir
    import concourse.tile as tile

    nc = bass.Bass(
        "TRN2",
        target_bir_lowering=False,
        debug=False,
        enable_asserts=False,
        num_devices=N_CORES,
    )
    pred = nc.dram_tensor(
        "pred", [P * FREE_TOTAL], mybir.dt.float32, kind="ExternalInput"
    )
    targ = nc.dram_tensor(
        "target", [P * FREE_TOTAL], mybir.dt.float32, kind="ExternalInput"
    )
    out = nc.dram_tensor("out", [P, 1], mybir.dt.float32, kind="ExternalOutput")

    def chunk_ap(t, c):
        w = CHUNK_WIDTHS[c]
        off = CHUNK_OFFS[c]
        return t.ap()[P * off : P * (off + w)].rearrange("(p w) -> p w", p=P)

    with tile.TileContext(nc) as tc:
        with (
            tc.tile_pool(name="io", bufs=1) as io_pool,
            tc.tile_pool(name="mid", bufs=1) as mid_pool,
            tc.tile_pool(name="accp", bufs=1) as acc_pool,
        ):
            d_tiles = []
            dma_handles = []
            for c in range(NCHUNK):
                d_c = io_pool.tile(
                    [P, CHUNK_WIDTHS[c]], mybir.dt.float32, tag=f"d{c}"
                )
                d_tiles.append(d_c)
            dma_handles.append(
                nc.gpsimd.dma_start(d_tiles[NCHUNK - 1][:], chunk_ap(pred, NCHUNK - 1))
            )
            for c in range(NCHUNK - 1):
                dma_handles.append(nc.sync.dma_start(d_tiles[c][:], chunk_ap(pred, c)))
            for c in range(NCHUNK):
                # -target accumulates onto pred in the DMA datapath (CCE add)
                dma_handles.append(
                    nc.gpsimd.dma_start(
                        d_tiles[c][:], chunk_ap(targ, c), accum_op=mybir.AluOpType.add
                    )
                )

            accs = []
            for c in range(NCHUNK):
                w = CHUNK_WIDTHS[c]
                # q = d^2 — squares alternate between ACT (even chunks) and
                # DVE (odd chunks) to balance engine busy time; both engines
                # then fit inside the DMA stream window.
                q = mid_pool.tile([P, w], mybir.dt.float32, tag=f"q{c}")
                if c % 2 == 0:
                    nc.scalar.square(q[:], d_tiles[c][:])
                else:
                    nc.vector.tensor_mul(q[:], d_tiles[c][:], d_tiles[c][:])

                # DVE: s = q_even + q_odd
                qv = q[:].rearrange("p (n two) -> p two n", two=2)
                s = mid_pool.tile([P, w // 2], mybir.dt.float32, tag=f"s{c}")
                nc.vector.tensor_add(s[:], qv[:, 0], qv[:, 1])

                # ACT: dist = sqrt(s) in place, accum partial
                acc_c = acc_pool.tile([P, 1], mybir.dt.float32, tag=f"acc{c}")
                accs.append(acc_c)
                hsq = nc.scalar.activation(
                    s[:],
                    s[:],
                    mybir.ActivationFunctionType.Sqrt,
                    accum_out=acc_c[:],
                )

            # Partials for chunks 0..5 reduce early (off the critical path):
            # (a0+a1), (a2+a3), (a4+a5) -> pairwise -> rB. The tail is only
            # r3 = a6 + a7 (one ACT wait) and final = rB + r3 (DVE own wait);
            # every add reads either two ACT-written or two DVE-written
            # tiles, keeping each at a single sync wait.
            def dve_add(name, x, y):
                r = acc_pool.tile([P, 1], mybir.dt.float32, tag=name)
                h = nc.vector.tensor_add(r[:], x[:], y[:])
                return r, h

            r0, _ = dve_add("t_r0", accs[0], accs[1])
            r1, _ = dve_add("t_r1", accs[2], accs[3])
            r2, _ = dve_add("t_r2", accs[4], accs[5])
            rA, _ = dve_add("t_rA", r0, r1)
            rB, _ = dve_add("t_rB", rA, r2)
            r3, _ = dve_add("t_r3", accs[6], accs[7])
            tot, final_add = dve_add("tot", rB, r3)

            hout = nc.sync.dma_start(out.ap(), tot[:])

            # SP observer chain: the kernel-tail drain inherits a wait for
            # every proc SP hasn't observed, and it too is capped at one
            # wait. One reg_mov per outstanding completion elides the
            # drain's waits.
            with nc.sync.register("tailr") as rr:
                for h in dma_handles + [hsq, final_add, hout]:
                    hm = nc.sync.reg_mov(rr, 0)
                    tile.add_dep_helper(
                        hm.ins, h.ins, sync=True, reason="SP observes for tail drain"
                    )
    return nc


def _get_nc():
    if "nc" not in _NC_CACHE:
        _NC_CACHE["nc"] = _build()
    return _NC_CACHE["nc"]


def kernel(pred, target, **run_kwargs):
    global LAST_RESULTS
    from concourse.bass_utils import run_bass_kernel_spmd

    pred = np.ascontiguousarray(np.asarray(pred, dtype=np.float32))
    target = np.ascontiguousarray(np.asarray(target, dtype=np.float32))
    assert pred.shape == (B, 2) and target.shape == (B, 2)

    neg_target = -target
    in_maps = []
    for c in range(N_CORES):
        sl = slice(c * RPC, (c + 1) * RPC)
        in_maps.append(
            {
                "pred": pred[sl].reshape(-1),
                "target": neg_target[sl].reshape(-1),
            }
        )

    nc = _get_nc()
    results = run_bass_kernel_spmd(
        nc, in_maps, core_ids=list(range(N_CORES)), **run_kwargs
    )
    LAST_RESULTS = results

    total = np.float64(0.0)
    for r in results.results:
        total += r["out"].astype(np.float64).sum()
    loss = np.float32(total / np.float64(B + 1))
    return np.asarray(loss, dtype=np.float32)
